# revision 34
# baseline (speedup 1.0000x reference)
"""Bidirectional GINConv on 8 Trainium2 NeuronCores.

Strategy (dst-node sharding, zero collectives):
  - Pad node space to 50176 = 8 * 49 * 128; core k owns the 49 dst tiles
    (128 nodes each) of range [k*6272, (k+1)*6272).
  - Host groups edges (plus one synthetic self-edge per node, implementing
    the `x + agg` term) by (dst tile, src half, direction) and pads each
    bucket to a per-slot chunk count (max over the 8 cores, so the SPMD
    program is shape-uniform); src indices become int16-local offsets.
  - Device: for each (tile, half) one `dma_gather` fetches x rows (encoded
    as bf16 hi|lo pairs, 512B/row) for BOTH directions; a DVE is_equal
    against an iota builds one-hot dst matrices; the PE accumulates
    agg = sum(onehot^T @ x) into PSUM (hi and lo column blocks side by
    side -> exact-ish fp32 after one add).
  - agg(hi)+agg(lo) -> h, PE-transpose, then the 2-layer MLP per direction,
    directions summed in PSUM, final relu((a+b)/2 + b2) on ACT, store.
  - Host concatenates the per-core [128, 6272] outputs and transposes.
"""

import sys

import numpy as np
import ml_dtypes

sys.path.insert(0, "/opt/trn_rl_repo")

P = 128
D = 128
N_NODES = 50000
N_EDGES = 800000
N_CORES = 8
TILES_PER_CORE = 49
NODES_PER_CORE = TILES_PER_CORE * P      # 6272
TABLE_ROWS = N_CORES * NODES_PER_CORE    # 50176
HALF = TABLE_ROWS // 2                   # 25088

_BF16 = ml_dtypes.bfloat16
assert True  # ROW_MODE "hilo" no longer supported (aggT operand swap)
SINGLE_PACKET = False  # True crashes the runtime (verified)
# "f16": x rows stored as single fp16 (256B gathers, ~3e-4 rel err)
# "hilo": x rows stored as bf16 hi|lo pairs (512B gathers, ~2e-6 rel err)
ROW_MODE = "f16"
# dev-only ablations for benching: subset of {"no_gather", "no_compute"}
ABLATE = set()
# "dma_gather": int16 gather, x table split in halves (2 groups)
# "indirect": indirect_dma_start with int32 offsets, single group
GATHER_MODE = "dma_gather"
# sort each bucket's edges by src id -> ascending HBM addresses per DMA
# engine stream (row-buffer locality)
SORT_SRC = True
# dma_gather ucode runs each op on ONE Q7 core pair picked by queue_num
# (cpu_id/2 == queue_num); round-robin over 4 queues -> 4x descriptor
# emission parallelism (8 cores instead of 2)
N_SWDGE_QUEUES = 4
# USE_QUEUES must divide the 8 DMASW sem lanes: lane k's consecutive
# gathers must stay on one queue (per-queue FIFO completion) or false
# cross-queue waits serialize the pipeline (3 queues measured WORSE).
USE_QUEUES = 4
# pad tail of dir-1 idx region with -1 so the Q7 trims it at runtime.
# Trim only at whole-128-chunk granularity: a non-multiple-of-128 trimmed
# count triggers the ucode's partial-chunk dummy-descriptor path, which
# crashes this runtime (verified empirically).
TRIM_PAD = False


def _n_groups():
    return 2 if GATHER_MODE == "dma_gather" else 1


def _bucketize(src, dst, n_tiles_total, ng):
    """Group (src, dst) edges by (dst tile[, src half])."""
    s = np.asarray(src, np.int64)
    t = np.asarray(dst, np.int64)
    tile_id = t >> 7
    grp = (s >= HALF).astype(np.int64) if ng == 2 else np.zeros_like(s)
    key = tile_id * ng + grp
    if SORT_SRC:
        # secondary sort by src id within each bucket
        order = np.lexsort((s, key))
    else:
        order = np.argsort(key, kind="stable")
    s_s = s[order]
    dl_s = (t[order] & 127)
    key_s = key[order]
    counts = np.bincount(key_s, minlength=n_tiles_total * ng)
    return s_s, dl_s, key_s, counts


def _host_prep(x, edge_index, reverse_edge_index):
    """Build per-core device input arrays.

    Returns (ch_slot, idx_cores, dstv_cores, xt, iota) where
    ch_slot[tl][g][d] = chunk count for that slot (uniform across cores).
    """
    ng = _n_groups()
    icu = 8 if GATHER_MODE == "dma_gather" else 1  # idx cols per chunk
    n_tiles_total = N_CORES * TILES_PER_CORE  # 392

    packed = []
    for ei in (edge_index, reverse_edge_index):
        packed.append(_bucketize(ei[0], ei[1], n_tiles_total, ng))

    # per-slot chunk counts: max over cores
    ch_slot = np.zeros((TILES_PER_CORE, ng, 2), np.int64)  # [tl, g, d]
    for d, (_, _, _, counts) in enumerate(packed):
        c = counts.reshape(N_CORES, TILES_PER_CORE, ng)  # [core, tl, grp]
        ch_slot[:, :, d] = np.maximum(
            ch_slot[:, :, d], -(-c.max(axis=0) // P))

    toti = int(ch_slot.sum()) * icu  # idx cols per core
    totd = int(ch_slot.sum())        # dstv chunk-cols per core
    # column offsets per (tile, group, dir)
    idx_off = np.zeros((TILES_PER_CORE, ng, 2), np.int64)
    dstv_off = np.zeros((TILES_PER_CORE, ng, 2), np.int64)
    acc = 0
    for tl in range(TILES_PER_CORE):
        for g in range(ng):
            for d in (0, 1):
                idx_off[tl, g, d] = acc * icu
                dstv_off[tl, g, d] = acc
                acc += int(ch_slot[tl, g, d])

    mdt_np = np.float16 if ROW_MODE == "f16" else _BF16
    idt_np = np.int16 if GATHER_MODE == "dma_gather" else np.int32
    idx_cores = np.zeros((N_CORES, P, toti), idt_np)
    dstv_cores = np.full((N_CORES, P, 2 * totd), -1.0, mdt_np)

    for d, (s_s, dl_s, key_s, counts) in enumerate(packed):
        offs = np.zeros(n_tiles_total * ng + 1, dtype=np.int64)
        np.cumsum(counts, out=offs[1:])
        for b in range(n_tiles_total * ng):
            n = int(counts[b])
            if n == 0:
                continue
            tile, g = divmod(b, ng)
            core, tl = divmod(tile, TILES_PER_CORE)
            cap = int(ch_slot[tl, g, d]) * P
            o = offs[b]
            io = idx_off[tl, g, d]
            if GATHER_MODE == "dma_gather":
                src_l = np.zeros(cap, np.int16)
                src_l[:n] = (s_s[o:o + n] - g * HALF).astype(np.int16)
                if d == 1 and TRIM_PAD:
                    # dir-1 region is the tail of the (tile, half) gather;
                    # trailing -1 idxs are trimmed by the Q7 at runtime
                    # (per-core actual counts, not the 8-core max). Keep the
                    # partial chunk 0-padded; only whole empty chunks get -1.
                    n128 = -(-n // P) * P
                    src_l[n128:] = -1
                # slot i -> [i % 16, i // 16], replicated to 8 groups of 16
                iw = src_l.reshape(cap // 16, 16).T  # [16, cap//16]
                idx_cores[core, :, io:io + cap // 16] = np.tile(iw, (8, 1))
            else:
                src_l = np.zeros(cap, np.int32)
                src_l[:n] = s_s[o:o + n].astype(np.int32)
                # slot i -> [i % 128, i // 128]
                idx_cores[core, :, io:io + cap // P] = \
                    src_l.reshape(cap // P, P).T
            dl = np.full(cap, -1.0, mdt_np)
            dl[:n] = dl_s[o:o + n].astype(np.float32).astype(mdt_np)
            # dstv: slot i -> [i % 128, i // 128], each value duplicated into
            # an adjacent pair so the one-hot is_equal reads it with a
            # unit-stride 2-element last dim (DVE 2x_1P mode)
            do = dstv_off[tl, g, d]
            dw = dl.reshape(cap // P, P).T  # [128, chunks]
            dstv_cores[core, :, 2 * do:2 * (do + cap // P)] = \
                np.repeat(dw, 2, axis=1)

    x = np.asarray(x, np.float32)
    if ROW_MODE == "f16":
        # x table: [TABLE_ROWS, 128] fp16 rows
        xt = np.zeros((TABLE_ROWS, D), np.float16)
        xt[:N_NODES] = x.astype(np.float16)
        mdt = np.float16
    else:
        # x table: [TABLE_ROWS, 256] bf16, row = [hi(128) | lo(128)]
        xt = np.zeros((TABLE_ROWS, 2 * D), _BF16)
        hi = x.astype(_BF16)
        lo = (x - hi.astype(np.float32)).astype(_BF16)
        xt[:N_NODES, :D] = hi
        xt[:N_NODES, D:] = lo
        mdt = _BF16

    ch_max = int(ch_slot.sum(axis=2).max())  # widest (tile, half) gather
    iota = np.tile(np.arange(P, dtype=np.float32),
                   (P, ch_max, 1)).astype(mdt)

    # exact f32 x, sharded by core, TRANSPOSED to [D, nodes] (adds the GIN
    # "+x" term without gather; transposed so agg can accumulate as
    # aggT[feat, dst] and the MLP needs no PE transpose)
    xf = np.zeros((TABLE_ROWS, D), np.float32)
    xf[:N_NODES] = x
    xf_cores = xf.reshape(N_CORES, NODES_PER_CORE, D)
    xft_cores = np.ascontiguousarray(xf_cores.transpose(0, 2, 1))
    return ch_slot, idx_off, dstv_off, toti, totd, idx_cores, dstv_cores, \
        xt, iota, ch_max, xft_cores


def _build_program(ch_slot, idx_off, dstv_off, toti, totd, ch_max,
                   n_tiles, enable_asserts=False, bench_reps=0):
    import contextlib
    from concourse import bacc, mybir
    import concourse.tile as tile
    from concourse.masks import make_identity

    dt = mybir.dt
    if ROW_MODE == "f16":
        mdt = dt.float16
        xw = D
    else:
        mdt = dt.bfloat16
        xw = 2 * D

    nc = bacc.Bacc(
        "TRN2",
        target_bir_lowering=False,
        debug=False,
        enable_asserts=enable_asserts,
        num_devices=1,
        # 32KB scratch -> 512-desc rings: two ~145-desc gathers fit per
        # ring, so the NX can emit group n+1 while group n drains
        dynamic_dma_scratch_size=32768,
        **({"num_swdge_queues": N_SWDGE_QUEUES} if N_SWDGE_QUEUES > 1 else {}),
    )

    idt = dt.int16 if GATHER_MODE == "dma_gather" else dt.int32
    xt = nc.dram_tensor(
        "xt", [TABLE_ROWS, xw], mdt, kind="ExternalInput").ap()
    idx = nc.dram_tensor(
        "idx", [P, toti], idt, kind="ExternalInput").ap()
    dstv = nc.dram_tensor(
        "dstv", [P, 2 * totd], mdt, kind="ExternalInput").ap()
    iotar = nc.dram_tensor(
        "iotar", [P, ch_max, P], mdt, kind="ExternalInput").ap()
    w1t = nc.dram_tensor(
        "w1t", [D, D], dt.float32, kind="ExternalInput").ap()
    w2t = nc.dram_tensor(
        "w2t", [D, D], dt.float32, kind="ExternalInput").ap()
    b1c = nc.dram_tensor(
        "b1c", [D, 1], dt.float32, kind="ExternalInput").ap()
    b2c = nc.dram_tensor(
        "b2c", [D, 1], dt.float32, kind="ExternalInput").ap()
    xf = nc.dram_tensor(
        "xf", [D, NODES_PER_CORE], dt.float32, kind="ExternalInput").ap()
    y = nc.dram_tensor(
        "y", [D, n_tiles * P], dt.float32, kind="ExternalOutput").ap()

    with tile.TileContext(nc) as tc:
        with (
            tc.tile_pool(name="const", bufs=1) as cpool,
            tc.tile_pool(name="xgp", bufs=14) as xgpool,
            tc.tile_pool(name="mp", bufs=14) as mpool,
            tc.tile_pool(name="fp", bufs=12) as fpool,
            tc.tile_pool(name="aggps", bufs=4, space="PSUM") as aggpool,
            tc.tile_pool(name="mlpps", bufs=4, space="PSUM") as mlppool,
        ):
            iota_sb = cpool.tile([P, ch_max, P], mdt)
            nc.sync.dma_start(out=iota_sb[:], in_=iotar[:])
            # idx/dstv resident in SBUF: two big line-rate DMAs replace ~200
            # per-tile small-descriptor loads (and their dependency chains)
            idx_all = cpool.tile([P, toti], idt)
            nc.sync.dma_start(out=idx_all[:], in_=idx[:])
            dstv_all = cpool.tile([P, 2 * totd], mdt)
            nc.sync.dma_start(out=dstv_all[:], in_=dstv[:])
            w1t_sb = cpool.tile([D, D], dt.float32)
            nc.sync.dma_start(out=w1t_sb[:], in_=w1t[:])
            w2t_sb = cpool.tile([D, D], dt.float32)
            nc.sync.dma_start(out=w2t_sb[:], in_=w2t[:])
            b1_sb = cpool.tile([D, 1], dt.float32)
            nc.sync.dma_start(out=b1_sb[:], in_=b1c[:])
            b2_sb = cpool.tile([D, 1], dt.float32)
            nc.sync.dma_start(out=b2_sb[:], in_=b2c[:])
            ident = cpool.tile([P, P], dt.float32)
            make_identity(nc, ident[:])

            loop_cm = (tc.For_i(0, bench_reps, 1) if bench_reps
                       else contextlib.nullcontext())
            with loop_cm:
                _build_tiles(
                    nc, tc, mybir, dt, mdt, idt, xw, n_tiles, ch_slot,
                    idx_off, dstv_off, idx_all, dstv_all, xf, y, xt, iota_sb,
                    w1t_sb, w2t_sb, b1_sb, b2_sb, ident, xgpool,
                    mpool, fpool, aggpool, mlppool)

    nc.compile()
    return nc


def _build_tiles(nc, tc, mybir, dt, mdt, idt, xw, n_tiles, ch_slot, idx_off,
                 dstv_off, idx_all, dstv_all, xf, y, xt, iota_sb, w1t_sb,
                 w2t_sb, b1_sb, b2_sb, ident, xgpool, mpool, fpool,
                 aggpool, mlppool):
    ng = _n_groups()
    gather_ctr = 0
    if True:
            for t in range(n_tiles):
                xf_sb = fpool.tile([D, P], dt.float32, tag="xf")
                nc.sync.dma_start(
                    out=xf_sb[:], in_=xf[:, t * P:(t + 1) * P])

                # per-(half, dir) gather: ~73 descs/engine-ring each, so
                # 3 fit in a 256-desc ring and the NX's await_space stops
                # serializing groups. One-hot built per (half) over both
                # dirs (dstv region is contiguous across dirs).
                xg_hd = {}
                m_h = {}
                for h in range(ng):
                    chs = int(ch_slot[t, h, 0] + ch_slot[t, h, 1])
                    if chs == 0:
                        continue
                    for d in (0, 1):
                        chs_d = int(ch_slot[t, h, d])
                        if chs_d == 0:
                            continue
                        io = int(idx_off[t, h, d])
                        xg = xgpool.tile([P, chs_d, xw], mdt, tag="xg")
                        if "no_gather" not in ABLATE:
                            nc.gpsimd.dma_gather(
                                out_ap=xg[:],
                                in_ap=xt[h * HALF:(h + 1) * HALF, :],
                                idxs_ap=idx_all[:, io:io + chs_d * 8],
                                num_idxs=chs_d * P,
                                num_idxs_reg=chs_d * P,
                                elem_size=xw,
                                single_packet=SINGLE_PACKET,
                                queue_num=gather_ctr % USE_QUEUES,
                            )
                            gather_ctr += 1
                        xg_hd[(h, d)] = xg
                    if "no_compute" in ABLATE:
                        continue
                    do = int(dstv_off[t, h, 0])
                    m_sb = mpool.tile([P, chs, P], mdt, tag="m")
                    # all operands get a unit-stride 2-element last dim so
                    # the DVE runs is_equal in 2x_1P mode: dstv is stored as
                    # duplicated pairs, iota/out are viewed as [.., 64, 2]
                    nc.vector.tensor_tensor(
                        out=m_sb[:].rearrange(
                            "p c (j two) -> p c j two", two=2),
                        in0=dstv_all[:, 2 * do:2 * (do + chs)]
                        .rearrange("p (c two) -> p c two", two=2)
                        [:, :, None, :].to_broadcast([P, chs, P // 2, 2]),
                        in1=iota_sb[:, :chs, :].rearrange(
                            "p c (j two) -> p c j two", two=2),
                        op=mybir.AluOpType.is_equal,
                    )
                    m_h[h] = m_sb

                if "no_compute" in ABLATE:
                    continue
                # agg accumulated TRANSPOSED: aggT[feat, dst] = sum_chunks
                # xg[e, feat]^T @ onehot[e, dst] -> the MLP consumes hT
                # directly, no PE transpose needed.
                r1_tiles = []
                for d in (0, 1):
                    agg_ps = aggpool.tile([P, P], dt.float32, tag="agg")
                    # chunk list: (half, local chunk in xg, chunk in m)
                    chunks = []
                    for h in range(ng):
                        base = 0 if d == 0 else int(ch_slot[t, h, 0])
                        for c in range(int(ch_slot[t, h, d])):
                            chunks.append((h, c, base + c))
                    for i, (h, c, mc) in enumerate(chunks):
                        nc.tensor.matmul(
                            out=agg_ps[:],
                            lhsT=xg_hd[(h, d)][:, c, :],
                            rhs=m_h[h][:, mc, :],
                            start=(i == 0),
                            stop=(i == len(chunks) - 1),
                        )
                    ht_sb = fpool.tile([D, P], dt.float32, tag="ht")
                    if not chunks:
                        nc.vector.tensor_copy(out=ht_sb[:], in_=xf_sb[:])
                    else:
                        nc.vector.tensor_tensor(
                            out=ht_sb[:], in0=xf_sb[:], in1=agg_ps[:],
                            op=mybir.AluOpType.add)
                    l1_ps = mlppool.tile([P, D], dt.float32, tag="mlp")
                    nc.tensor.matmul(
                        out=l1_ps[:], lhsT=w1t_sb[:], rhs=ht_sb[:],
                        start=True, stop=True)
                    r1_sb = fpool.tile([P, D], dt.float32, tag="r1")
                    nc.scalar.activation(
                        out=r1_sb[:], in_=l1_ps[:],
                        func=mybir.ActivationFunctionType.Relu,
                        bias=b1_sb[:], scale=1.0)
                    r1_tiles.append(r1_sb)

                l2_ps = mlppool.tile([P, D], dt.float32, tag="mlp")
                nc.tensor.matmul(
                    out=l2_ps[:], lhsT=w2t_sb[:], rhs=r1_tiles[0][:],
                    start=True, stop=False)
                nc.tensor.matmul(
                    out=l2_ps[:], lhsT=w2t_sb[:], rhs=r1_tiles[1][:],
                    start=False, stop=True)
                out_sb = fpool.tile([P, D], dt.float32, tag="out")
                nc.scalar.activation(
                    out=out_sb[:], in_=l2_ps[:],
                    func=mybir.ActivationFunctionType.Relu,
                    bias=b2_sb[:], scale=0.5)
                nc.sync.dma_start(
                    out=y[:, t * P:(t + 1) * P], in_=out_sb[:])


_CACHE = {}
_LAST = {}


def _get_program(ch_slot, idx_off, dstv_off, toti, totd, ch_max):
    key = (tuple(ch_slot.ravel()), TILES_PER_CORE)
    if key not in _CACHE:
        _CACHE[key] = _build_program(
            ch_slot, idx_off, dstv_off, toti, totd, ch_max, TILES_PER_CORE)
    return _CACHE[key]


def kernel(x, edge_index, reverse_edge_index, w1, b1, w2, b2):
    from concourse.bass_utils import run_bass_kernel_spmd

    (ch_slot, idx_off, dstv_off, toti, totd, idx_cores, dstv_cores,
     xt, iota, ch_max, xft_cores) = _host_prep(
        x, edge_index, reverse_edge_index)
    nc = _get_program(ch_slot, idx_off, dstv_off, toti, totd, ch_max)

    w1t = np.ascontiguousarray(np.asarray(w1, np.float32).T)
    w2t = np.ascontiguousarray(np.asarray(w2, np.float32).T)
    b1c = np.ascontiguousarray(np.asarray(b1, np.float32)[:, None])
    b2c = np.ascontiguousarray(np.asarray(b2, np.float32)[:, None])

    in_maps = []
    for k in range(N_CORES):
        in_maps.append({
            "xt": xt,
            "idx": idx_cores[k],
            "dstv": dstv_cores[k],
            "iotar": iota,
            "w1t": w1t,
            "w2t": w2t,
            "b1c": b1c,
            "b2c": b2c,
            "xf": np.ascontiguousarray(xft_cores[k]),
        })

    res = run_bass_kernel_spmd(nc, in_maps, list(range(N_CORES)))
    _LAST["res"] = res
    y = np.concatenate([res.results[k]["y"] for k in range(N_CORES)], axis=1)
    return np.ascontiguousarray(y.T[:N_NODES])



# revision 38
# speedup vs baseline: 1.0395x; 1.0395x over previous
"""Bidirectional GINConv on 8 Trainium2 NeuronCores.

Strategy (dst-node sharding, zero collectives):
  - Pad node space to 50176 = 8 * 49 * 128; core k owns the 49 dst tiles
    (128 nodes each) of range [k*6272, (k+1)*6272).
  - Host groups edges (plus one synthetic self-edge per node, implementing
    the `x + agg` term) by (dst tile, src half, direction) and pads each
    bucket to a per-slot chunk count (max over the 8 cores, so the SPMD
    program is shape-uniform); src indices become int16-local offsets.
  - Device: for each (tile, half) one `dma_gather` fetches x rows (encoded
    as bf16 hi|lo pairs, 512B/row) for BOTH directions; a DVE is_equal
    against an iota builds one-hot dst matrices; the PE accumulates
    agg = sum(onehot^T @ x) into PSUM (hi and lo column blocks side by
    side -> exact-ish fp32 after one add).
  - agg(hi)+agg(lo) -> h, PE-transpose, then the 2-layer MLP per direction,
    directions summed in PSUM, final relu((a+b)/2 + b2) on ACT, store.
  - Host concatenates the per-core [128, 6272] outputs and transposes.
"""

import sys

import numpy as np
import ml_dtypes

sys.path.insert(0, "/opt/trn_rl_repo")

P = 128
D = 128
N_NODES = 50000
N_EDGES = 800000
N_CORES = 8
TILES_PER_CORE = 49
NODES_PER_CORE = TILES_PER_CORE * P      # 6272
TABLE_ROWS = N_CORES * NODES_PER_CORE    # 50176
HALF = TABLE_ROWS // 2                   # 25088

_BF16 = ml_dtypes.bfloat16
assert True  # ROW_MODE "hilo" no longer supported (aggT operand swap)
SINGLE_PACKET = False  # True crashes the runtime (verified)
# "f16": x rows stored as single fp16 (256B gathers, ~3e-4 rel err)
# "hilo": x rows stored as bf16 hi|lo pairs (512B gathers, ~2e-6 rel err)
ROW_MODE = "f16"
# dev-only ablations for benching: subset of {"no_gather", "no_compute"}
ABLATE = set()
# "dma_gather": int16 gather, x table split in halves (2 groups)
# "indirect": indirect_dma_start with int32 offsets, single group
GATHER_MODE = "dma_gather"
# sort each bucket's edges by src id -> ascending HBM addresses per DMA
# engine stream (row-buffer locality)
SORT_SRC = True
# dma_gather ucode runs each op on ONE Q7 core pair picked by queue_num
# (cpu_id/2 == queue_num); round-robin over 4 queues -> 4x descriptor
# emission parallelism (8 cores instead of 2)
N_SWDGE_QUEUES = 4
# USE_QUEUES must divide the 8 DMASW sem lanes: lane k's consecutive
# gathers must stay on one queue (per-queue FIFO completion) or false
# cross-queue waits serialize the pipeline (3 queues measured WORSE).
USE_QUEUES = 4
# pad tail of dir-1 idx region with -1 so the Q7 trims it at runtime.
# Trim only at whole-128-chunk granularity: a non-multiple-of-128 trimmed
# count triggers the ucode's partial-chunk dummy-descriptor path, which
# crashes this runtime (verified empirically).
TRIM_PAD = False


def _n_groups():
    return 2 if GATHER_MODE == "dma_gather" else 1


def _bucketize(src, dst, n_tiles_total, ng):
    """Group (src, dst) edges by (dst tile[, src half])."""
    s = np.asarray(src, np.int64)
    t = np.asarray(dst, np.int64)
    tile_id = t >> 7
    grp = (s >= HALF).astype(np.int64) if ng == 2 else np.zeros_like(s)
    key = tile_id * ng + grp
    if SORT_SRC:
        # secondary sort by src id within each bucket
        order = np.lexsort((s, key))
    else:
        order = np.argsort(key, kind="stable")
    s_s = s[order]
    dl_s = (t[order] & 127)
    key_s = key[order]
    counts = np.bincount(key_s, minlength=n_tiles_total * ng)
    return s_s, dl_s, key_s, counts


def _host_prep(x, edge_index, reverse_edge_index):
    """Build per-core device input arrays.

    Returns (ch_slot, idx_cores, dstv_cores, xt, iota) where
    ch_slot[tl][g][d] = chunk count for that slot (uniform across cores).
    """
    ng = _n_groups()
    icu = 8 if GATHER_MODE == "dma_gather" else 1  # idx cols per chunk
    n_tiles_total = N_CORES * TILES_PER_CORE  # 392

    packed = []
    for ei in (edge_index, reverse_edge_index):
        packed.append(_bucketize(ei[0], ei[1], n_tiles_total, ng))

    # per-slot chunk counts: max over cores
    ch_slot = np.zeros((TILES_PER_CORE, ng, 2), np.int64)  # [tl, g, d]
    for d, (_, _, _, counts) in enumerate(packed):
        c = counts.reshape(N_CORES, TILES_PER_CORE, ng)  # [core, tl, grp]
        ch_slot[:, :, d] = np.maximum(
            ch_slot[:, :, d], -(-c.max(axis=0) // P))

    toti = int(ch_slot.sum()) * icu  # idx cols per core
    totd = int(ch_slot.sum())        # dstv chunk-cols per core
    # column offsets per (tile, group, dir)
    idx_off = np.zeros((TILES_PER_CORE, ng, 2), np.int64)
    dstv_off = np.zeros((TILES_PER_CORE, ng, 2), np.int64)
    acc = 0
    for tl in range(TILES_PER_CORE):
        for g in range(ng):
            for d in (0, 1):
                idx_off[tl, g, d] = acc * icu
                dstv_off[tl, g, d] = acc
                acc += int(ch_slot[tl, g, d])

    mdt_np = np.float16 if ROW_MODE == "f16" else _BF16
    idt_np = np.int16 if GATHER_MODE == "dma_gather" else np.int32
    idx_cores = np.zeros((N_CORES, P, toti), idt_np)
    dstv_cores = np.full((N_CORES, P, 2 * totd), -1.0, mdt_np)

    for d, (s_s, dl_s, key_s, counts) in enumerate(packed):
        offs = np.zeros(n_tiles_total * ng + 1, dtype=np.int64)
        np.cumsum(counts, out=offs[1:])
        for b in range(n_tiles_total * ng):
            n = int(counts[b])
            if n == 0:
                continue
            tile, g = divmod(b, ng)
            core, tl = divmod(tile, TILES_PER_CORE)
            cap = int(ch_slot[tl, g, d]) * P
            o = offs[b]
            io = idx_off[tl, g, d]
            if GATHER_MODE == "dma_gather":
                src_l = np.zeros(cap, np.int16)
                src_l[:n] = (s_s[o:o + n] - g * HALF).astype(np.int16)
                if d == 1 and TRIM_PAD:
                    # dir-1 region is the tail of the (tile, half) gather;
                    # trailing -1 idxs are trimmed by the Q7 at runtime
                    # (per-core actual counts, not the 8-core max). Keep the
                    # partial chunk 0-padded; only whole empty chunks get -1.
                    n128 = -(-n // P) * P
                    src_l[n128:] = -1
                # slot i -> [i % 16, i // 16], replicated to 8 groups of 16
                iw = src_l.reshape(cap // 16, 16).T  # [16, cap//16]
                idx_cores[core, :, io:io + cap // 16] = np.tile(iw, (8, 1))
            else:
                src_l = np.zeros(cap, np.int32)
                src_l[:n] = s_s[o:o + n].astype(np.int32)
                # slot i -> [i % 128, i // 128]
                idx_cores[core, :, io:io + cap // P] = \
                    src_l.reshape(cap // P, P).T
            dl = np.full(cap, -1.0, mdt_np)
            dl[:n] = dl_s[o:o + n].astype(np.float32).astype(mdt_np)
            # dstv: slot i -> [i % 128, i // 128], each value duplicated into
            # an adjacent pair so the one-hot is_equal reads it with a
            # unit-stride 2-element last dim (DVE 2x_1P mode)
            do = dstv_off[tl, g, d]
            dw = dl.reshape(cap // P, P).T  # [128, chunks]
            dstv_cores[core, :, 2 * do:2 * (do + cap // P)] = \
                np.repeat(dw, 2, axis=1)

    x = np.asarray(x, np.float32)
    if ROW_MODE == "f16":
        # x table: [TABLE_ROWS, 128] fp16 rows
        xt = np.zeros((TABLE_ROWS, D), np.float16)
        xt[:N_NODES] = x.astype(np.float16)
        mdt = np.float16
    else:
        # x table: [TABLE_ROWS, 256] bf16, row = [hi(128) | lo(128)]
        xt = np.zeros((TABLE_ROWS, 2 * D), _BF16)
        hi = x.astype(_BF16)
        lo = (x - hi.astype(np.float32)).astype(_BF16)
        xt[:N_NODES, :D] = hi
        xt[:N_NODES, D:] = lo
        mdt = _BF16

    ch_max = int(ch_slot.sum(axis=2).max())  # widest (tile, half) gather
    iota = np.tile(np.arange(P, dtype=np.float32),
                   (P, ch_max, 1)).astype(mdt)

    # exact f32 x, sharded by core, TRANSPOSED to [D, nodes] (adds the GIN
    # "+x" term without gather; transposed so agg can accumulate as
    # aggT[feat, dst] and the MLP needs no PE transpose)
    xf = np.zeros((TABLE_ROWS, D), np.float32)
    xf[:N_NODES] = x
    xf_cores = xf.reshape(N_CORES, NODES_PER_CORE, D)
    xft_cores = np.ascontiguousarray(xf_cores.transpose(0, 2, 1))
    return ch_slot, idx_off, dstv_off, toti, totd, idx_cores, dstv_cores, \
        xt, iota, ch_max, xft_cores


def _build_program(ch_slot, idx_off, dstv_off, toti, totd, ch_max,
                   n_tiles, enable_asserts=False, bench_reps=0):
    import contextlib
    from concourse import bacc, mybir
    import concourse.tile as tile
    from concourse.masks import make_identity

    dt = mybir.dt
    if ROW_MODE == "f16":
        mdt = dt.float16
        xw = D
    else:
        mdt = dt.bfloat16
        xw = 2 * D

    nc = bacc.Bacc(
        "TRN2",
        target_bir_lowering=False,
        debug=False,
        enable_asserts=enable_asserts,
        num_devices=1,
        # 32KB scratch -> 512-desc rings: two ~145-desc gathers fit per
        # ring, so the NX can emit group n+1 while group n drains
        dynamic_dma_scratch_size=32768,
        **({"num_swdge_queues": N_SWDGE_QUEUES} if N_SWDGE_QUEUES > 1 else {}),
    )

    idt = dt.int16 if GATHER_MODE == "dma_gather" else dt.int32
    xt = nc.dram_tensor(
        "xt", [TABLE_ROWS, xw], mdt, kind="ExternalInput").ap()
    idx = nc.dram_tensor(
        "idx", [P, toti], idt, kind="ExternalInput").ap()
    dstv = nc.dram_tensor(
        "dstv", [P, 2 * totd], mdt, kind="ExternalInput").ap()
    iotar = nc.dram_tensor(
        "iotar", [P, ch_max, P], mdt, kind="ExternalInput").ap()
    w1t = nc.dram_tensor(
        "w1t", [D, D], dt.float32, kind="ExternalInput").ap()
    w2t = nc.dram_tensor(
        "w2t", [D, D], dt.float32, kind="ExternalInput").ap()
    b1c = nc.dram_tensor(
        "b1c", [D, 1], dt.float32, kind="ExternalInput").ap()
    b2c = nc.dram_tensor(
        "b2c", [D, 1], dt.float32, kind="ExternalInput").ap()
    xf = nc.dram_tensor(
        "xf", [D, NODES_PER_CORE], dt.float32, kind="ExternalInput").ap()
    y = nc.dram_tensor(
        "y", [D, n_tiles * P], dt.float32, kind="ExternalOutput").ap()

    with tile.TileContext(nc) as tc:
        with (
            tc.tile_pool(name="const", bufs=1) as cpool,
            tc.tile_pool(name="xgp", bufs=14) as xgpool,
            tc.tile_pool(name="mp", bufs=14) as mpool,
            tc.tile_pool(name="fp", bufs=12) as fpool,
            tc.tile_pool(name="aggps", bufs=4, space="PSUM") as aggpool,
            tc.tile_pool(name="mlpps", bufs=4, space="PSUM") as mlppool,
        ):
            # idx/dstv resident in SBUF: two big line-rate DMAs replace ~200
            # per-tile small-descriptor loads (and their dependency chains).
            # idx_all FIRST: the first gather only needs it, so the gather
            # pipeline starts while the other constants still load.
            idx_all = cpool.tile([P, toti], idt)
            nc.sync.dma_start(out=idx_all[:], in_=idx[:])
            dstv_all = cpool.tile([P, 2 * totd], mdt)
            nc.sync.dma_start(out=dstv_all[:], in_=dstv[:])
            iota_sb = cpool.tile([P, ch_max, P], mdt)
            nc.sync.dma_start(out=iota_sb[:], in_=iotar[:])
            w1t_sb = cpool.tile([D, D], dt.float32)
            nc.sync.dma_start(out=w1t_sb[:], in_=w1t[:])
            w2t_sb = cpool.tile([D, D], dt.float32)
            nc.sync.dma_start(out=w2t_sb[:], in_=w2t[:])
            b1_sb = cpool.tile([D, 1], dt.float32)
            nc.sync.dma_start(out=b1_sb[:], in_=b1c[:])
            b2_sb = cpool.tile([D, 1], dt.float32)
            nc.sync.dma_start(out=b2_sb[:], in_=b2c[:])
            ident = cpool.tile([P, P], dt.float32)
            make_identity(nc, ident[:])

            loop_cm = (tc.For_i(0, bench_reps, 1) if bench_reps
                       else contextlib.nullcontext())
            with loop_cm:
                _build_tiles(
                    nc, tc, mybir, dt, mdt, idt, xw, n_tiles, ch_slot,
                    idx_off, dstv_off, idx_all, dstv_all, xf, y, xt, iota_sb,
                    w1t_sb, w2t_sb, b1_sb, b2_sb, ident, xgpool,
                    mpool, fpool, aggpool, mlppool)

    nc.compile()
    return nc


def _build_tiles(nc, tc, mybir, dt, mdt, idt, xw, n_tiles, ch_slot, idx_off,
                 dstv_off, idx_all, dstv_all, xf, y, xt, iota_sb, w1t_sb,
                 w2t_sb, b1_sb, b2_sb, ident, xgpool, mpool, fpool,
                 aggpool, mlppool):
    ng = _n_groups()
    gather_ctr = 0
    if True:
            for t in range(n_tiles):
                xf_sb = fpool.tile([D, P], dt.float32, tag="xf")
                nc.sync.dma_start(
                    out=xf_sb[:], in_=xf[:, t * P:(t + 1) * P])

                # per-half gather (covers both dirs; per-dir split measured
                # WORSE: doubles per-gather fixed costs) + one-hot build
                xg_hd = {}
                m_h = {}
                for h in range(ng):
                    chs = int(ch_slot[t, h, 0] + ch_slot[t, h, 1])
                    if chs == 0:
                        continue
                    io = int(idx_off[t, h, 0])
                    xg = xgpool.tile([P, chs, xw], mdt, tag="xg")
                    if "no_gather" not in ABLATE:
                        nc.gpsimd.dma_gather(
                            out_ap=xg[:],
                            in_ap=xt[h * HALF:(h + 1) * HALF, :],
                            idxs_ap=idx_all[:, io:io + chs * 8],
                            num_idxs=chs * P,
                            num_idxs_reg=chs * P,
                            elem_size=xw,
                            single_packet=SINGLE_PACKET,
                            queue_num=gather_ctr % USE_QUEUES,
                        )
                        gather_ctr += 1
                    xg_hd[h] = xg
                    if "no_compute" in ABLATE:
                        continue
                    do = int(dstv_off[t, h, 0])
                    m_sb = mpool.tile([P, chs, P], mdt, tag="m")
                    # all operands get a unit-stride 2-element last dim so
                    # the DVE runs is_equal in 2x_1P mode: dstv is stored as
                    # duplicated pairs, iota/out are viewed as [.., 64, 2]
                    nc.vector.tensor_tensor(
                        out=m_sb[:].rearrange(
                            "p c (j two) -> p c j two", two=2),
                        in0=dstv_all[:, 2 * do:2 * (do + chs)]
                        .rearrange("p (c two) -> p c two", two=2)
                        [:, :, None, :].to_broadcast([P, chs, P // 2, 2]),
                        in1=iota_sb[:, :chs, :].rearrange(
                            "p c (j two) -> p c j two", two=2),
                        op=mybir.AluOpType.is_equal,
                    )
                    m_h[h] = m_sb

                if "no_compute" in ABLATE:
                    continue
                # agg accumulated TRANSPOSED: aggT[feat, dst] = sum_chunks
                # xg[e, feat]^T @ onehot[e, dst] -> the MLP consumes hT
                # directly, no PE transpose needed.
                r1_tiles = []
                for d in (0, 1):
                    agg_ps = aggpool.tile([P, P], dt.float32, tag="agg")
                    # chunk list: (half, local chunk in xg, chunk in m)
                    chunks = []
                    for h in range(ng):
                        base = 0 if d == 0 else int(ch_slot[t, h, 0])
                        for c in range(int(ch_slot[t, h, d])):
                            chunks.append((h, c, base + c))
                    for i, (h, c, mc) in enumerate(chunks):
                        nc.tensor.matmul(
                            out=agg_ps[:],
                            lhsT=xg_hd[h][:, mc, :],
                            rhs=m_h[h][:, mc, :],
                            start=(i == 0),
                            stop=(i == len(chunks) - 1),
                        )
                    ht_sb = fpool.tile([D, P], dt.float32, tag="ht")
                    if not chunks:
                        nc.vector.tensor_copy(out=ht_sb[:], in_=xf_sb[:])
                    else:
                        nc.vector.tensor_tensor(
                            out=ht_sb[:], in0=xf_sb[:], in1=agg_ps[:],
                            op=mybir.AluOpType.add)
                    l1_ps = mlppool.tile([P, D], dt.float32, tag="mlp")
                    nc.tensor.matmul(
                        out=l1_ps[:], lhsT=w1t_sb[:], rhs=ht_sb[:],
                        start=True, stop=True)
                    r1_sb = fpool.tile([P, D], dt.float32, tag="r1")
                    nc.scalar.activation(
                        out=r1_sb[:], in_=l1_ps[:],
                        func=mybir.ActivationFunctionType.Relu,
                        bias=b1_sb[:], scale=1.0)
                    r1_tiles.append(r1_sb)

                l2_ps = mlppool.tile([P, D], dt.float32, tag="mlp")
                nc.tensor.matmul(
                    out=l2_ps[:], lhsT=w2t_sb[:], rhs=r1_tiles[0][:],
                    start=True, stop=False)
                nc.tensor.matmul(
                    out=l2_ps[:], lhsT=w2t_sb[:], rhs=r1_tiles[1][:],
                    start=False, stop=True)
                out_sb = fpool.tile([P, D], dt.float32, tag="out")
                nc.scalar.activation(
                    out=out_sb[:], in_=l2_ps[:],
                    func=mybir.ActivationFunctionType.Relu,
                    bias=b2_sb[:], scale=0.5)
                nc.sync.dma_start(
                    out=y[:, t * P:(t + 1) * P], in_=out_sb[:])


_CACHE = {}
_LAST = {}


def _get_program(ch_slot, idx_off, dstv_off, toti, totd, ch_max):
    key = (tuple(ch_slot.ravel()), TILES_PER_CORE)
    if key not in _CACHE:
        _CACHE[key] = _build_program(
            ch_slot, idx_off, dstv_off, toti, totd, ch_max, TILES_PER_CORE)
    return _CACHE[key]


def kernel(x, edge_index, reverse_edge_index, w1, b1, w2, b2):
    from concourse.bass_utils import run_bass_kernel_spmd

    (ch_slot, idx_off, dstv_off, toti, totd, idx_cores, dstv_cores,
     xt, iota, ch_max, xft_cores) = _host_prep(
        x, edge_index, reverse_edge_index)
    nc = _get_program(ch_slot, idx_off, dstv_off, toti, totd, ch_max)

    w1t = np.ascontiguousarray(np.asarray(w1, np.float32).T)
    w2t = np.ascontiguousarray(np.asarray(w2, np.float32).T)
    b1c = np.ascontiguousarray(np.asarray(b1, np.float32)[:, None])
    b2c = np.ascontiguousarray(np.asarray(b2, np.float32)[:, None])

    in_maps = []
    for k in range(N_CORES):
        in_maps.append({
            "xt": xt,
            "idx": idx_cores[k],
            "dstv": dstv_cores[k],
            "iotar": iota,
            "w1t": w1t,
            "w2t": w2t,
            "b1c": b1c,
            "b2c": b2c,
            "xf": np.ascontiguousarray(xft_cores[k]),
        })

    res = run_bass_kernel_spmd(nc, in_maps, list(range(N_CORES)))
    _LAST["res"] = res
    y = np.concatenate([res.results[k]["y"] for k in range(N_CORES)], axis=1)
    return np.ascontiguousarray(y.T[:N_NODES])



# revision 39
# speedup vs baseline: 1.0469x; 1.0071x over previous
"""Bidirectional GINConv on 8 Trainium2 NeuronCores.

Strategy (dst-node sharding, zero collectives):
  - Pad node space to 50176 = 8 * 49 * 128; core k owns the 49 dst tiles
    (128 nodes each) of range [k*6272, (k+1)*6272).
  - Host groups edges (plus one synthetic self-edge per node, implementing
    the `x + agg` term) by (dst tile, src half, direction) and pads each
    bucket to a per-slot chunk count (max over the 8 cores, so the SPMD
    program is shape-uniform); src indices become int16-local offsets.
  - Device: for each (tile, half) one `dma_gather` fetches x rows (encoded
    as bf16 hi|lo pairs, 512B/row) for BOTH directions; a DVE is_equal
    against an iota builds one-hot dst matrices; the PE accumulates
    agg = sum(onehot^T @ x) into PSUM (hi and lo column blocks side by
    side -> exact-ish fp32 after one add).
  - agg(hi)+agg(lo) -> h, PE-transpose, then the 2-layer MLP per direction,
    directions summed in PSUM, final relu((a+b)/2 + b2) on ACT, store.
  - Host concatenates the per-core [128, 6272] outputs and transposes.
"""

import sys

import numpy as np
import ml_dtypes

sys.path.insert(0, "/opt/trn_rl_repo")

P = 128
D = 128
N_NODES = 50000
N_EDGES = 800000
N_CORES = 8
TILES_PER_CORE = 49
NODES_PER_CORE = TILES_PER_CORE * P      # 6272
TABLE_ROWS = N_CORES * NODES_PER_CORE    # 50176
HALF = TABLE_ROWS // 2                   # 25088

_BF16 = ml_dtypes.bfloat16
assert True  # ROW_MODE "hilo" no longer supported (aggT operand swap)
SINGLE_PACKET = False  # True crashes the runtime (verified)
# "f16": x rows stored as single fp16 (256B gathers, ~3e-4 rel err)
# "hilo": x rows stored as bf16 hi|lo pairs (512B gathers, ~2e-6 rel err)
ROW_MODE = "f16"
# dev-only ablations for benching: subset of {"no_gather", "no_compute"}
ABLATE = set()
# "dma_gather": int16 gather, x table split in halves (2 groups)
# "indirect": indirect_dma_start with int32 offsets, single group
GATHER_MODE = "dma_gather"
# sort each bucket's edges by src id -> ascending HBM addresses per DMA
# engine stream (row-buffer locality)
SORT_SRC = True
# dma_gather ucode runs each op on ONE Q7 core pair picked by queue_num
# (cpu_id/2 == queue_num); round-robin over 4 queues -> 4x descriptor
# emission parallelism (8 cores instead of 2)
N_SWDGE_QUEUES = 4
# USE_QUEUES must divide the 8 DMASW sem lanes: lane k's consecutive
# gathers must stay on one queue (per-queue FIFO completion) or false
# cross-queue waits serialize the pipeline (3 queues measured WORSE).
USE_QUEUES = 4
# pad tail of dir-1 idx region with -1 so the Q7 trims it at runtime.
# Trim only at whole-128-chunk granularity: a non-multiple-of-128 trimmed
# count triggers the ucode's partial-chunk dummy-descriptor path, which
# crashes this runtime (verified empirically).
TRIM_PAD = False


def _n_groups():
    return 2 if GATHER_MODE == "dma_gather" else 1


def _bucketize(src, dst, n_tiles_total, ng):
    """Group (src, dst) edges by (dst tile[, src half])."""
    s = np.asarray(src, np.int64)
    t = np.asarray(dst, np.int64)
    tile_id = t >> 7
    grp = (s >= HALF).astype(np.int64) if ng == 2 else np.zeros_like(s)
    key = tile_id * ng + grp
    if SORT_SRC:
        # secondary sort by src id within each bucket
        order = np.lexsort((s, key))
    else:
        order = np.argsort(key, kind="stable")
    s_s = s[order]
    dl_s = (t[order] & 127)
    key_s = key[order]
    counts = np.bincount(key_s, minlength=n_tiles_total * ng)
    return s_s, dl_s, key_s, counts


def _host_prep(x, edge_index, reverse_edge_index):
    """Build per-core device input arrays.

    Returns (ch_slot, idx_cores, dstv_cores, xt, iota) where
    ch_slot[tl][g][d] = chunk count for that slot (uniform across cores).
    """
    ng = _n_groups()
    icu = 8 if GATHER_MODE == "dma_gather" else 1  # idx cols per chunk
    n_tiles_total = N_CORES * TILES_PER_CORE  # 392

    packed = []
    for ei in (edge_index, reverse_edge_index):
        packed.append(_bucketize(ei[0], ei[1], n_tiles_total, ng))

    # per-slot chunk counts: max over cores
    ch_slot = np.zeros((TILES_PER_CORE, ng, 2), np.int64)  # [tl, g, d]
    for d, (_, _, _, counts) in enumerate(packed):
        c = counts.reshape(N_CORES, TILES_PER_CORE, ng)  # [core, tl, grp]
        ch_slot[:, :, d] = np.maximum(
            ch_slot[:, :, d], -(-c.max(axis=0) // P))

    toti = int(ch_slot.sum()) * icu  # idx cols per core
    totd = int(ch_slot.sum())        # dstv chunk-cols per core
    # column offsets per (tile, group, dir)
    idx_off = np.zeros((TILES_PER_CORE, ng, 2), np.int64)
    dstv_off = np.zeros((TILES_PER_CORE, ng, 2), np.int64)
    acc = 0
    for tl in range(TILES_PER_CORE):
        for g in range(ng):
            for d in (0, 1):
                idx_off[tl, g, d] = acc * icu
                dstv_off[tl, g, d] = acc
                acc += int(ch_slot[tl, g, d])

    mdt_np = np.float16 if ROW_MODE == "f16" else _BF16
    idt_np = np.int16 if GATHER_MODE == "dma_gather" else np.int32
    idx_cores = np.zeros((N_CORES, P, toti), idt_np)
    dstv_cores = np.full((N_CORES, P, 2 * totd), -1.0, mdt_np)

    for d, (s_s, dl_s, key_s, counts) in enumerate(packed):
        offs = np.zeros(n_tiles_total * ng + 1, dtype=np.int64)
        np.cumsum(counts, out=offs[1:])
        for b in range(n_tiles_total * ng):
            n = int(counts[b])
            if n == 0:
                continue
            tile, g = divmod(b, ng)
            core, tl = divmod(tile, TILES_PER_CORE)
            cap = int(ch_slot[tl, g, d]) * P
            o = offs[b]
            io = idx_off[tl, g, d]
            if GATHER_MODE == "dma_gather":
                src_l = np.zeros(cap, np.int16)
                src_l[:n] = (s_s[o:o + n] - g * HALF).astype(np.int16)
                if d == 1 and TRIM_PAD:
                    # dir-1 region is the tail of the (tile, half) gather;
                    # trailing -1 idxs are trimmed by the Q7 at runtime
                    # (per-core actual counts, not the 8-core max). Keep the
                    # partial chunk 0-padded; only whole empty chunks get -1.
                    n128 = -(-n // P) * P
                    src_l[n128:] = -1
                # slot i -> [i % 16, i // 16], replicated to 8 groups of 16
                iw = src_l.reshape(cap // 16, 16).T  # [16, cap//16]
                idx_cores[core, :, io:io + cap // 16] = np.tile(iw, (8, 1))
            else:
                src_l = np.zeros(cap, np.int32)
                src_l[:n] = s_s[o:o + n].astype(np.int32)
                # slot i -> [i % 128, i // 128]
                idx_cores[core, :, io:io + cap // P] = \
                    src_l.reshape(cap // P, P).T
            dl = np.full(cap, -1.0, mdt_np)
            dl[:n] = dl_s[o:o + n].astype(np.float32).astype(mdt_np)
            # dstv: slot i -> [i % 128, i // 128], each value duplicated into
            # an adjacent pair so the one-hot is_equal reads it with a
            # unit-stride 2-element last dim (DVE 2x_1P mode)
            do = dstv_off[tl, g, d]
            dw = dl.reshape(cap // P, P).T  # [128, chunks]
            dstv_cores[core, :, 2 * do:2 * (do + cap // P)] = \
                np.repeat(dw, 2, axis=1)

    x = np.asarray(x, np.float32)
    if ROW_MODE == "f16":
        # x table: [TABLE_ROWS, 128] fp16 rows
        xt = np.zeros((TABLE_ROWS, D), np.float16)
        xt[:N_NODES] = x.astype(np.float16)
        mdt = np.float16
    else:
        # x table: [TABLE_ROWS, 256] bf16, row = [hi(128) | lo(128)]
        xt = np.zeros((TABLE_ROWS, 2 * D), _BF16)
        hi = x.astype(_BF16)
        lo = (x - hi.astype(np.float32)).astype(_BF16)
        xt[:N_NODES, :D] = hi
        xt[:N_NODES, D:] = lo
        mdt = _BF16

    ch_max = int(ch_slot.sum(axis=2).max())  # widest (tile, half) gather
    iota = np.tile(np.arange(P, dtype=np.float32),
                   (P, ch_max, 1)).astype(mdt)

    # exact f32 x, sharded by core, TRANSPOSED to [D, nodes] (adds the GIN
    # "+x" term without gather; transposed so agg can accumulate as
    # aggT[feat, dst] and the MLP needs no PE transpose)
    xf = np.zeros((TABLE_ROWS, D), np.float32)
    xf[:N_NODES] = x
    xf_cores = xf.reshape(N_CORES, NODES_PER_CORE, D)
    xft_cores = np.ascontiguousarray(xf_cores.transpose(0, 2, 1))
    return ch_slot, idx_off, dstv_off, toti, totd, idx_cores, dstv_cores, \
        xt, iota, ch_max, xft_cores


def _build_program(ch_slot, idx_off, dstv_off, toti, totd, ch_max,
                   n_tiles, enable_asserts=False, bench_reps=0):
    import contextlib
    from concourse import bacc, mybir
    import concourse.tile as tile
    from concourse.masks import make_identity

    dt = mybir.dt
    if ROW_MODE == "f16":
        mdt = dt.float16
        xw = D
    else:
        mdt = dt.bfloat16
        xw = 2 * D

    nc = bacc.Bacc(
        "TRN2",
        target_bir_lowering=False,
        debug=False,
        enable_asserts=enable_asserts,
        num_devices=1,
        # 32KB scratch -> 512-desc rings: two ~145-desc gathers fit per
        # ring, so the NX can emit group n+1 while group n drains
        dynamic_dma_scratch_size=32768,
        **({"num_swdge_queues": N_SWDGE_QUEUES} if N_SWDGE_QUEUES > 1 else {}),
    )

    idt = dt.int16 if GATHER_MODE == "dma_gather" else dt.int32
    xt = nc.dram_tensor(
        "xt", [TABLE_ROWS, xw], mdt, kind="ExternalInput").ap()
    idx = nc.dram_tensor(
        "idx", [P, toti], idt, kind="ExternalInput").ap()
    dstv = nc.dram_tensor(
        "dstv", [P, 2 * totd], mdt, kind="ExternalInput").ap()
    iotar = nc.dram_tensor(
        "iotar", [P, ch_max, P], mdt, kind="ExternalInput").ap()
    w1t = nc.dram_tensor(
        "w1t", [D, D], dt.float32, kind="ExternalInput").ap()
    w2t = nc.dram_tensor(
        "w2t", [D, D], dt.float32, kind="ExternalInput").ap()
    b1c = nc.dram_tensor(
        "b1c", [D, 1], dt.float32, kind="ExternalInput").ap()
    b2c = nc.dram_tensor(
        "b2c", [D, 1], dt.float32, kind="ExternalInput").ap()
    xf = nc.dram_tensor(
        "xf", [D, NODES_PER_CORE], dt.float32, kind="ExternalInput").ap()
    y = nc.dram_tensor(
        "y", [D, n_tiles * P], dt.float32, kind="ExternalOutput").ap()

    with tile.TileContext(nc) as tc:
        with (
            tc.tile_pool(name="const", bufs=1) as cpool,
            tc.tile_pool(name="xgp", bufs=14) as xgpool,
            tc.tile_pool(name="mp", bufs=14) as mpool,
            tc.tile_pool(name="fp", bufs=12) as fpool,
            tc.tile_pool(name="aggps", bufs=4, space="PSUM") as aggpool,
            tc.tile_pool(name="mlpps", bufs=4, space="PSUM") as mlppool,
        ):
            # idx/dstv resident in SBUF: two big line-rate DMAs replace ~200
            # per-tile small-descriptor loads (and their dependency chains).
            # idx_all FIRST: the first gather only needs it, so the gather
            # pipeline starts while the other constants still load.
            idx_all = cpool.tile([P, toti], idt)
            nc.sync.dma_start(out=idx_all[:], in_=idx[:])
            dstv_all = cpool.tile([P, 2 * totd], mdt)
            nc.sync.dma_start(out=dstv_all[:], in_=dstv[:])
            iota_sb = cpool.tile([P, ch_max, P], mdt)
            nc.sync.dma_start(out=iota_sb[:], in_=iotar[:])
            w1t_sb = cpool.tile([D, D], dt.float32)
            nc.sync.dma_start(out=w1t_sb[:], in_=w1t[:])
            w2t_sb = cpool.tile([D, D], dt.float32)
            nc.sync.dma_start(out=w2t_sb[:], in_=w2t[:])
            b1_sb = cpool.tile([D, 1], dt.float32)
            nc.sync.dma_start(out=b1_sb[:], in_=b1c[:])
            b2_sb = cpool.tile([D, 1], dt.float32)
            nc.sync.dma_start(out=b2_sb[:], in_=b2c[:])
            ident = cpool.tile([P, P], dt.float32)
            make_identity(nc, ident[:])

            loop_cm = (tc.For_i(0, bench_reps, 1) if bench_reps
                       else contextlib.nullcontext())
            with loop_cm:
                _build_tiles(
                    nc, tc, mybir, dt, mdt, idt, xw, n_tiles, ch_slot,
                    idx_off, dstv_off, idx_all, dstv_all, xf, y, xt, iota_sb,
                    w1t_sb, w2t_sb, b1_sb, b2_sb, ident, xgpool,
                    mpool, fpool, aggpool, mlppool)

    nc.compile()
    return nc


def _build_tiles(nc, tc, mybir, dt, mdt, idt, xw, n_tiles, ch_slot, idx_off,
                 dstv_off, idx_all, dstv_all, xf, y, xt, iota_sb, w1t_sb,
                 w2t_sb, b1_sb, b2_sb, ident, xgpool, mpool, fpool,
                 aggpool, mlppool):
    ng = _n_groups()
    # Greedy tile ordering to balance per-queue descriptor totals: the
    # kernel is emission-rate-bound per Q7 core pair, so the slowest queue
    # sets the span. Position i sends half-0 to queue 2i%4 and half-1 to
    # queue (2i+1)%4; pick the remaining tile minimizing the running max.
    loads = [0.0] * USE_QUEUES
    remaining = set(range(n_tiles))
    tile_order = []
    ctr_sim = 0
    while remaining:
        qa = ctr_sim % USE_QUEUES
        qb = (ctr_sim + 1) % USE_QUEUES
        best, best_val = None, None
        for cand in remaining:
            l0 = float(ch_slot[cand, 0, :].sum())
            l1 = float(ch_slot[cand, 1, :].sum()) if ng == 2 else 0.0
            trial = loads.copy()
            trial[qa] += l0
            trial[qb] += l1
            val = (max(trial), -(l0 + l1))
            if best_val is None or val < best_val:
                best, best_val = cand, val
        tile_order.append(best)
        remaining.discard(best)
        l0 = float(ch_slot[best, 0, :].sum())
        l1 = float(ch_slot[best, 1, :].sum()) if ng == 2 else 0.0
        loads[qa] += l0
        if l0 > 0:
            ctr_sim += 1
        loads[qb if l0 > 0 else qa] += l1
        if l1 > 0:
            ctr_sim += 1
    gather_ctr = 0
    if True:
            for t in tile_order:
                xf_sb = fpool.tile([D, P], dt.float32, tag="xf")
                nc.sync.dma_start(
                    out=xf_sb[:], in_=xf[:, t * P:(t + 1) * P])

                # per-half gather (covers both dirs; per-dir split measured
                # WORSE: doubles per-gather fixed costs) + one-hot build
                xg_hd = {}
                m_h = {}
                for h in range(ng):
                    chs = int(ch_slot[t, h, 0] + ch_slot[t, h, 1])
                    if chs == 0:
                        continue
                    io = int(idx_off[t, h, 0])
                    xg = xgpool.tile([P, chs, xw], mdt, tag="xg")
                    if "no_gather" not in ABLATE:
                        nc.gpsimd.dma_gather(
                            out_ap=xg[:],
                            in_ap=xt[h * HALF:(h + 1) * HALF, :],
                            idxs_ap=idx_all[:, io:io + chs * 8],
                            num_idxs=chs * P,
                            num_idxs_reg=chs * P,
                            elem_size=xw,
                            single_packet=SINGLE_PACKET,
                            queue_num=gather_ctr % USE_QUEUES,
                        )
                        gather_ctr += 1
                    xg_hd[h] = xg
                    if "no_compute" in ABLATE:
                        continue
                    do = int(dstv_off[t, h, 0])
                    m_sb = mpool.tile([P, chs, P], mdt, tag="m")
                    # all operands get a unit-stride 2-element last dim so
                    # the DVE runs is_equal in 2x_1P mode: dstv is stored as
                    # duplicated pairs, iota/out are viewed as [.., 64, 2]
                    nc.vector.tensor_tensor(
                        out=m_sb[:].rearrange(
                            "p c (j two) -> p c j two", two=2),
                        in0=dstv_all[:, 2 * do:2 * (do + chs)]
                        .rearrange("p (c two) -> p c two", two=2)
                        [:, :, None, :].to_broadcast([P, chs, P // 2, 2]),
                        in1=iota_sb[:, :chs, :].rearrange(
                            "p c (j two) -> p c j two", two=2),
                        op=mybir.AluOpType.is_equal,
                    )
                    m_h[h] = m_sb

                if "no_compute" in ABLATE:
                    continue
                # agg accumulated TRANSPOSED: aggT[feat, dst] = sum_chunks
                # xg[e, feat]^T @ onehot[e, dst] -> the MLP consumes hT
                # directly, no PE transpose needed.
                r1_tiles = []
                for d in (0, 1):
                    agg_ps = aggpool.tile([P, P], dt.float32, tag="agg")
                    # chunk list: (half, local chunk in xg, chunk in m)
                    chunks = []
                    for h in range(ng):
                        base = 0 if d == 0 else int(ch_slot[t, h, 0])
                        for c in range(int(ch_slot[t, h, d])):
                            chunks.append((h, c, base + c))
                    for i, (h, c, mc) in enumerate(chunks):
                        nc.tensor.matmul(
                            out=agg_ps[:],
                            lhsT=xg_hd[h][:, mc, :],
                            rhs=m_h[h][:, mc, :],
                            start=(i == 0),
                            stop=(i == len(chunks) - 1),
                        )
                    ht_sb = fpool.tile([D, P], dt.float32, tag="ht")
                    if not chunks:
                        nc.vector.tensor_copy(out=ht_sb[:], in_=xf_sb[:])
                    else:
                        nc.vector.tensor_tensor(
                            out=ht_sb[:], in0=xf_sb[:], in1=agg_ps[:],
                            op=mybir.AluOpType.add)
                    l1_ps = mlppool.tile([P, D], dt.float32, tag="mlp")
                    nc.tensor.matmul(
                        out=l1_ps[:], lhsT=w1t_sb[:], rhs=ht_sb[:],
                        start=True, stop=True)
                    r1_sb = fpool.tile([P, D], dt.float32, tag="r1")
                    nc.scalar.activation(
                        out=r1_sb[:], in_=l1_ps[:],
                        func=mybir.ActivationFunctionType.Relu,
                        bias=b1_sb[:], scale=1.0)
                    r1_tiles.append(r1_sb)

                l2_ps = mlppool.tile([P, D], dt.float32, tag="mlp")
                nc.tensor.matmul(
                    out=l2_ps[:], lhsT=w2t_sb[:], rhs=r1_tiles[0][:],
                    start=True, stop=False)
                nc.tensor.matmul(
                    out=l2_ps[:], lhsT=w2t_sb[:], rhs=r1_tiles[1][:],
                    start=False, stop=True)
                out_sb = fpool.tile([P, D], dt.float32, tag="out")
                nc.scalar.activation(
                    out=out_sb[:], in_=l2_ps[:],
                    func=mybir.ActivationFunctionType.Relu,
                    bias=b2_sb[:], scale=0.5)
                nc.sync.dma_start(
                    out=y[:, t * P:(t + 1) * P], in_=out_sb[:])


_CACHE = {}
_LAST = {}


def _get_program(ch_slot, idx_off, dstv_off, toti, totd, ch_max):
    key = (tuple(ch_slot.ravel()), TILES_PER_CORE)
    if key not in _CACHE:
        _CACHE[key] = _build_program(
            ch_slot, idx_off, dstv_off, toti, totd, ch_max, TILES_PER_CORE)
    return _CACHE[key]


def kernel(x, edge_index, reverse_edge_index, w1, b1, w2, b2):
    from concourse.bass_utils import run_bass_kernel_spmd

    (ch_slot, idx_off, dstv_off, toti, totd, idx_cores, dstv_cores,
     xt, iota, ch_max, xft_cores) = _host_prep(
        x, edge_index, reverse_edge_index)
    nc = _get_program(ch_slot, idx_off, dstv_off, toti, totd, ch_max)

    w1t = np.ascontiguousarray(np.asarray(w1, np.float32).T)
    w2t = np.ascontiguousarray(np.asarray(w2, np.float32).T)
    b1c = np.ascontiguousarray(np.asarray(b1, np.float32)[:, None])
    b2c = np.ascontiguousarray(np.asarray(b2, np.float32)[:, None])

    in_maps = []
    for k in range(N_CORES):
        in_maps.append({
            "xt": xt,
            "idx": idx_cores[k],
            "dstv": dstv_cores[k],
            "iotar": iota,
            "w1t": w1t,
            "w2t": w2t,
            "b1c": b1c,
            "b2c": b2c,
            "xf": np.ascontiguousarray(xft_cores[k]),
        })

    res = run_bass_kernel_spmd(nc, in_maps, list(range(N_CORES)))
    _LAST["res"] = res
    y = np.concatenate([res.results[k]["y"] for k in range(N_CORES)], axis=1)
    return np.ascontiguousarray(y.T[:N_NODES])

